# revision 4
# baseline (speedup 1.0000x reference)
"""Trainium2 Bass kernel: FlowNet-style local correlation (9x9 window) + softmax.

Computes, for inputs x,y [B=4, C=1024, H=96, W=96]:
  q = conv1x1(y; query_w, query_b) / 256   # [B, 256, H, W]  (scale folded)
  k = conv1x1(x; key_w,  key_b)            # [B, 256, H, W]
  corr[b,h,w,di,dj] = sum_c q[b,c,h,w] * kpad[b,c,h+di,w+dj]
  out = softmax(corr over the 81 (di,dj) channels)  # [B, H, W, 81]

Sharding: 8 cores = 4 batches x 2 H-halves (48 rows each, 4-row halo on the
k side, handled by host-side zero padding + row-masked key bias).

Per-core kernel (v3):
  - SWDGE cast-DMA loads (fp32 HBM -> bf16 SBUF)
  - q projection: 9 groups of 512 flat (h,w) columns, K=1024 in 8
    PSUM-accumulated chunks; bias + 1/256 scale folded (weights pre-scaled
    on host, bias applied on ScalarE evacuation)
  - k projection: 14 groups of 4 rows (N=384); row-masked bias applied on
    VectorE evacuation.  Emission interleaves projection groups and
    correlation rows so TensorE stays dense under the DMA envelope.
  - correlation per output row h: lhsT = q[:, h, :], rhs = 9 consecutive
    padded k rows -> [128, 936] into one 2-bank PSUM tile (N=512 + N=424),
    accumulated over the 2 C4 chunks.
  - evacuation (alternating ScalarE/VectorE per row) applies a free strided
    transpose: score col (di*104 + wp) -> (wp*9 + di).  In that order, the
    DRAM shear (write pitch 936, read pitch 945) makes the 81 band values of
    every pixel CONTIGUOUS: band read is a plain 2D [96, 81] DMA per row.
  - softmax per 12-row block: ScalarE exp, VectorE tensor_reduce + recip +
    broadcast-mul (the mul also permutes (dj,di) -> (di,dj) channel order),
    per-row [96, 81] output DMAs.
"""

import numpy as np

import concourse.bacc as bacc
import concourse.bass as bass
import concourse.mybir as mybir
import concourse.tile as tile
from concourse.bass_utils import run_bass_kernel_spmd

F32 = mybir.dt.float32
BF16 = mybir.dt.bfloat16
AF = mybir.ActivationFunctionType

B, C, H, W = 4, 1024, 96, 96
C4 = 256
D = 4                # max displacement
ND = 2 * D + 1       # 9
NB = ND * ND         # 81
HH = H // 2          # 48 rows per core
KR = HH + 2 * D      # 56 k rows incl. halo/pad
WP = W + 2 * D       # 104 padded k width
CC = C // 128        # 8 contraction chunks
MC = C4 // 128       # 2 output-channel chunks
QN = 512             # q projection free dim per matmul
NQG = HH * W // QN   # 9 q groups
RG = 4               # k rows per projection group
NKG = KR // RG       # 14 k groups
SB = ND * WP         # 936 score columns per output row
NS1 = 512
NS2 = SB - NS1       # 424
RQ = 96 * (SB + ND)  # 90720: padded per-row region in DRAM scratch
HB = 12              # rows per softmax block
NBLK = HH // HB      # 4
N_CORES = 8


def _build_tile(tc, xs, ys, wqt, wkt, bqs, bkr, out):
    nc = tc.nc
    with (
        tc.tile_pool(name="const", bufs=1) as const,
        tc.tile_pool(name="big", bufs=1) as big,
        tc.tile_pool(name="stq", bufs=3) as stq_pool,
        tc.tile_pool(name="stk", bufs=3) as stk_pool,
        tc.tile_pool(name="erow", bufs=3) as erow_pool,
        tc.tile_pool(name="band", bufs=2) as band_pool,
        tc.tile_pool(name="soft", bufs=2) as soft_pool,
        tc.tile_pool(name="psq", bufs=4, space="PSUM") as psq,
        tc.tile_pool(name="psAB", bufs=2, space="PSUM") as psAB,
        tc.tile_pool(name="dram", bufs=NBLK, space="DRAM") as dram,
    ):
        # --- constants: weights (transposed + q pre-scaled on host), biases ---
        wq_sb = const.tile([128, CC, C4], BF16)
        nc.gpsimd.dma_start(wq_sb[:], wqt.rearrange("(cc p) o -> p cc o", p=128))
        wk_sb = const.tile([128, CC, C4], BF16)
        nc.gpsimd.dma_start(wk_sb[:], wkt.rearrange("(cc p) o -> p cc o", p=128))
        bq_sb = const.tile([128, MC], F32)
        nc.gpsimd.dma_start(bq_sb[:], bqs.rearrange("(m p) -> p m", p=128))
        bkr_sb = const.tile([128, MC, KR], F32)
        nc.gpsimd.dma_start(bkr_sb[:], bkr.rearrange("(m p) r -> p m r", p=128))

        q_sb = big.tile([128, MC, HH * W], BF16)
        k_sb = big.tile([128, MC, KR, WP], BF16)
        # only the left/right pad columns need zeroing: halo rows arrive
        # zeroed via the padded input + row-masked bias
        nc.vector.memset(k_sb[:, :, :, 0:D], 0.0)
        nc.vector.memset(k_sb[:, :, :, D + W:WP], 0.0)

        ys3 = ys.rearrange("(cc p) f -> p cc f", p=128)

        def emit_k_group(g):
            st = stk_pool.tile([128, CC, RG * W], BF16, tag="stk")
            nc.gpsimd.dma_start(
                st[:],
                xs[:, g * RG:(g + 1) * RG, :].rearrange(
                    "(cc p) h w -> p cc (h w)", p=128
                ),
            )
            for m in range(MC):
                ps = psq.tile([128, QN], F32, tag="psq")
                for cc in range(CC):
                    nc.tensor.matmul(
                        ps[:, 0:RG * W],
                        wk_sb[:, cc, m * 128:(m + 1) * 128],
                        st[:, cc, :],
                        start=(cc == 0),
                        stop=(cc == CC - 1),
                    )
                # k = Wk x + bk*rowmask (bias pre-masked per row on host so
                # halo/pad rows stay exactly zero, matching the zero-pad)
                bias3 = (
                    bkr_sb[:, m, g * RG:(g + 1) * RG]
                    .unsqueeze(-1)
                    .broadcast_to((128, RG, W))
                )
                nc.vector.tensor_tensor(
                    k_sb[:, m, g * RG:(g + 1) * RG, D:D + W],
                    ps[:, 0:RG * W].rearrange("p (a b) -> p a b", b=W),
                    bias3,
                    op=mybir.AluOpType.add,
                )

        def emit_q_group(g):
            st = stq_pool.tile([128, CC, QN], BF16, tag="stq")
            nc.gpsimd.dma_start(st[:], ys3[:, :, g * QN:(g + 1) * QN])
            for m in range(MC):
                ps = psq.tile([128, QN], F32, tag="psq")
                for cc in range(CC):
                    nc.tensor.matmul(
                        ps[:],
                        wq_sb[:, cc, m * 128:(m + 1) * 128],
                        st[:, cc, :],
                        start=(cc == 0),
                        stop=(cc == CC - 1),
                    )
                nc.scalar.activation(
                    q_sb[:, m, g * QN:(g + 1) * QN],
                    ps[:],
                    AF.Identity,
                    bias=bq_sb[:, m:m + 1],
                    scale=1.0,
                )

        sd_blks = {}
        band_blks = {}

        def emit_corr_row(h):
            blk, r = divmod(h, HB)
            if r == 0:
                sd_new = dram.tile([HB * RQ], BF16, tag="sd")
                sd_blks[blk] = sd_new
                band_new = band_pool.tile([96, HB, NB], BF16, tag="band")
                band_blks[blk] = band_new
            sd = sd_blks[blk]
            ps = psAB.tile([96, SB], F32, tag="psab")
            for m in range(MC):
                lhsT = q_sb[:, m, h * W:(h + 1) * W]
                rhs = k_sb[:, m, h:h + ND, :].rearrange("p a b -> p (a b)")
                nc.tensor.matmul(
                    ps[:, 0:NS1], lhsT, rhs[:, 0:NS1],
                    start=(m == 0), stop=(m == MC - 1),
                )
                nc.tensor.matmul(
                    ps[:, NS1:SB], lhsT, rhs[:, NS1:SB],
                    start=(m == 0), stop=(m == MC - 1),
                )
            # evacuate with (di, wp) -> (wp, di) column reorder so the DRAM
            # shear lands each pixel's 81 band values contiguously
            e = erow_pool.tile([96, SB], BF16, tag="e")
            src = ps[:].rearrange("p (di wp) -> p wp di", di=ND)
            dst = e[:].rearrange("p (wp di) -> p wp di", di=ND)
            if h % 2 == 0:
                nc.scalar.copy(dst, src)
            else:
                nc.vector.tensor_copy(dst, src)
            # write at pitch SB inside RQ-sized per-row regions; re-reading
            # at pitch SB+ND shears so band (w, dj*9+di) = row[w*945 + dj*9+di]
            wdst = (
                sd[:].rearrange("(r z) -> r z", z=RQ)[r:r + 1, 0:96 * SB]
                .rearrange("r (w c) -> (r w) c", c=SB)
            )
            nc.sync.dma_start(wdst, e[:])
            sheared = sd[:].rearrange("(r w c) -> r w c", w=96, c=SB + ND)
            nc.sync.dma_start(band_blks[blk][:, r, :], sheared[r, :, 0:NB])

        def emit_block(blk):
            sd_blks.pop(blk)
            band = band_blks.pop(blk)
            p = soft_pool.tile([96, HB, NB], F32, tag="p")
            nc.scalar.activation(
                p[:].rearrange("p a b -> p (a b)"),
                band[:].rearrange("p a b -> p (a b)"),
                AF.Exp,
            )
            ssum = soft_pool.tile([96, HB], F32, tag="ssum")
            nc.vector.tensor_reduce(
                ssum[:], p[:], axis=mybir.AxisListType.X, op=mybir.AluOpType.add
            )
            rinv = soft_pool.tile([96, HB], F32, tag="rinv")
            nc.vector.reciprocal(rinv[:], ssum[:])
            # normalize + permute band channel order (dj,di) -> (di,dj)
            o = soft_pool.tile([96, HB, NB], F32, tag="o")
            nc.vector.tensor_tensor(
                o[:].rearrange("p r (di dj) -> p r di dj", di=ND),
                p[:].rearrange("p r (dj di) -> p r di dj", di=ND),
                rinv[:].unsqueeze(-1).unsqueeze(-1)
                .broadcast_to((96, HB, ND, ND)),
                op=mybir.AluOpType.mult,
            )
            for r in range(HB):
                nc.sync.dma_start(out[blk * HB + r], o[:, r, :])

        # interleaved emission: keep TensorE fed while stage DMAs stream
        done_q = 0
        done_c = 0

        def drain(ready):
            nonlocal done_q, done_c
            while done_q < NQG and done_q * QN < ready * W:
                emit_q_group(done_q)
                done_q += 1
            while done_c < ready and (done_c + 1) * W <= done_q * QN:
                emit_corr_row(done_c)
                done_c += 1
                if done_c % HB == 0:
                    emit_block(done_c // HB - 1)

        for kg in range(NKG):
            emit_k_group(kg)
            drain(min(max(0, RG * (kg + 1) - ND + 1), HH))
        while done_q < NQG:
            emit_q_group(done_q)
            done_q += 1
        drain(HH)


def build_bass(debug_taps=False):
    nc = bacc.Bacc("TRN2", target_bir_lowering=False, debug=False,
                   num_devices=N_CORES)
    xs = nc.dram_tensor("xs", [C, KR, W], F32, kind="ExternalInput")
    ys = nc.dram_tensor("ys", [C, HH * W], F32, kind="ExternalInput")
    wqt = nc.dram_tensor("wqt", [C, C4], F32, kind="ExternalInput")
    wkt = nc.dram_tensor("wkt", [C, C4], F32, kind="ExternalInput")
    bqs = nc.dram_tensor("bqs", [C4], F32, kind="ExternalInput")
    bkr = nc.dram_tensor("bkr", [C4, KR], F32, kind="ExternalInput")
    out = nc.dram_tensor("out", [HH, W, NB], F32, kind="ExternalOutput")
    with tile.TileContext(nc) as tc:
        _build_tile(tc, xs.ap(), ys.ap(), wqt.ap(), wkt.ap(), bqs.ap(),
                    bkr.ap(), out.ap())
    nc.compile()
    return nc


def make_in_maps(x, y, query_w, query_b, key_w, key_b):
    x = np.asarray(x, dtype=np.float32)
    y = np.asarray(y, dtype=np.float32)
    xp = np.pad(x, ((0, 0), (0, 0), (D, D), (0, 0)))
    # fold the 1/C4 correlation normalization into the q projection
    wqt = np.ascontiguousarray(np.asarray(query_w, np.float32).T / C4)
    wkt = np.ascontiguousarray(np.asarray(key_w, np.float32).T)
    bqs = (np.asarray(query_b, np.float32) / C4).astype(np.float32)
    kb = np.asarray(key_b, np.float32)
    in_maps = []
    for core in range(N_CORES):
        b, half = divmod(core, 2)
        h0 = half * HH
        rows = np.arange(KR) + h0 - D
        mask = ((rows >= 0) & (rows < H)).astype(np.float32)
        in_maps.append({
            "xs": np.ascontiguousarray(xp[b, :, h0:h0 + KR, :]),
            "ys": np.ascontiguousarray(
                y[b, :, h0:h0 + HH, :].reshape(C, HH * W)),
            "wqt": wqt,
            "wkt": wkt,
            "bqs": bqs,
            "bkr": np.ascontiguousarray(kb[:, None] * mask[None, :]),
        })
    return in_maps


_NC = None


def _get_nc():
    global _NC
    if _NC is None:
        _NC = build_bass()
    return _NC


def kernel(x, y, query_w, query_b, key_w, key_b, _trace=False):
    nc = _get_nc()
    in_maps = make_in_maps(x, y, query_w, query_b, key_w, key_b)
    res = run_bass_kernel_spmd(nc, in_maps, core_ids=list(range(N_CORES)),
                               trace=_trace)
    out = np.empty((B, H, W, NB), np.float32)
    for core in range(N_CORES):
        b, half = divmod(core, 2)
        out[b, half * HH:(half + 1) * HH] = res.results[core]["out"]
    if _trace:
        kernel.last_results = res
    return out
